# revision 1
# baseline (speedup 1.0000x reference)
"""Trainium2 Bass kernel for nn_LsqNonneg: batched NNLS via 200-iteration FISTA.

Math (matches reference.py exactly, in fp32):
    AtA = A.T @ A                       [32,32]
    L   = ||AtA||_2,  step = 1/L
    B   = step * A.T @ X                [32, N]
    W   = I - step*AtA                  [32,32]
    S_1 = relu(B);  S_0 = 0
    for k = 1..199:
        mu_k   = (t_{k-1}-1)/t_k        (t_0=1, t_k = 0.5(1+sqrt(1+4 t_{k-1}^2)))
        S_{k+1} = relu( (1+mu_k) W S_k  -  mu_k W S_{k-1}  +  B )   # momentum folded
    return S_200

Device layout (per core, NS=4096 columns):
    S stored as [128, NS/4]: partition group g (rows 32g..32g+31) holds columns
    [g*NS/4, (g+1)*NS/4); so one [128, 512] slice carries 4 independent column
    blocks stacked in partitions.  Weights are block-diagonal diag4(W) [128,128]
    so a single full-array matmul advances 4 column blocks at once with a full
    128-wide PSUM drain (one bank per 512-col slice).
    Per iteration per 512-slice: 3 accumulating matmuls into one PSUM bank:
       ident128 @ B   (start=True: writes B)
       diag4((1+mu)W) @ S_cur   (accumulate)
       diag4((-mu)W)  @ S_prev  (accumulate, stop)
    then relu(psum) -> S buffer (VectorE on slice 0, ScalarE on slice 1 so the
    two PSUM banks are read in parallel).  Per-iteration folded weights are
    streamed from DRAM (double-buffered), precomputed on host.
"""

import os
import sys

import numpy as np

for _p in ("/opt/trn_rl_repo", "/root/.axon_site/_ro/trn_rl_repo"):
    if os.path.isdir(_p) and _p not in sys.path:
        sys.path.append(_p)

from contextlib import ExitStack

import concourse.bass as bass
import concourse.bacc as bacc
import concourse.tile as tile
from concourse import mybir
from concourse.bass_utils import run_bass_kernel_spmd

M, K, N_FULL, N_CORES, ITERS = 512, 32, 32768, 8, 200

F32 = mybir.dt.float32
F32R = mybir.dt.float32r
F16 = mybir.dt.float16

# mm dtype for the PE: float32r runs at 1 cycle/row (vs 4 for float32) but with
# reduced precision on hardware; measured empirically via test.py.
MM_DTYPE = F32R

LAST_RESULTS = None  # BassKernelResults of the most recent run (for test.py)


def _mm(ap, dt_):
    return ap.bitcast(dt_) if dt_ is not F32 else ap


def build_program(ns: int, iters: int, mm_dtype=MM_DTYPE):
    """Build the SPMD Bass program for one core holding `ns` columns."""
    DT = mm_dtype
    q = ns // 4          # free extent of the packed [128, q] S layout
    nsl = q // 512       # number of 512-wide slices (PSUM banks per generation)
    assert ns % 2048 == 0 and nsl >= 1

    nc = bacc.Bacc("TRN2", target_bir_lowering=False)

    x_d = nc.dram_tensor("x", [M, ns], F32, kind="ExternalInput")
    apad_d = nc.dram_tensor("apad", [4, M, 128], F32, kind="ExternalInput")
    wd_d = nc.dram_tensor("wd", [max(iters - 1, 1), 2, 128, 128], F32,
                          kind="ExternalInput")
    id_d = nc.dram_tensor("ident", [128, 128], F32, kind="ExternalInput")
    out_d = nc.dram_tensor("s_out", [K, ns], F32, kind="ExternalOutput")

    with ExitStack() as ctx:
        tc = ctx.enter_context(tile.TileContext(nc))
        persist = ctx.enter_context(tc.tile_pool(name="persist", bufs=1))
        xpool = ctx.enter_context(tc.tile_pool(name="xstage", bufs=4))
        wpool = ctx.enter_context(tc.tile_pool(name="wstage", bufs=6))
        psum = ctx.enter_context(tc.tile_pool(name="psum", bufs=3, space="PSUM"))

        s_a = persist.tile([128, q], DT)   # S_odd  generations
        s_b = persist.tile([128, q], DT)   # S_even generations
        b_sb = persist.tile([128, q], DT)  # B in packed layout
        id_sb = persist.tile([128, 128], DT)
        

        nc.sync.dma_start(id_sb[:], id_d[:].bitcast(DT))
        apc = persist.tile([128, 16 * 128], DT)  # (g,c) chunk at free 128*(4g+c)
        for g in range(4):
            for c in range(4):
                nc.sync.dma_start(
                    apc[:, 128 * (4 * g + c):128 * (4 * g + c + 1)],
                    apad_d[g, 128 * c:128 * (c + 1), :].bitcast(DT),
                )

        # ---- prologue: B = As.T @ X, packed layout, plus S_1 = relu(B) ----
        pb = psum.tile([128, q], F32, tag="pt")
        for c in range(4):
            xt = xpool.tile([128, ns], DT)
            nc.sync.dma_start(xt[:], x_d[128 * c:128 * (c + 1), :].bitcast(DT))
            for g in range(4):
                lhs = apc[:, 128 * (4 * g + c):128 * (4 * g + c + 1)]
                for s in range(nsl):
                    nc.tensor.matmul(
                        pb[:, 512 * s:512 * (s + 1)],
                        lhs,
                        xt[:, g * q + 512 * s: g * q + 512 * (s + 1)],
                        start=(c == 0 and g == 0),
                        stop=(c == 3 and g == 3),
                    )
        for s in range(nsl):
            sl = slice(512 * s, 512 * (s + 1))
            if s % 2 == 0:
                nc.vector.tensor_copy(b_sb[:, sl], pb[:, sl])
                nc.scalar.activation(s_a[:, sl], pb[:, sl],
                                     mybir.ActivationFunctionType.Relu)
            else:
                nc.scalar.copy(b_sb[:, sl], pb[:, sl])
                nc.vector.tensor_scalar_max(s_a[:, sl], pb[:, sl], 0.0)

        # ---- FISTA loop: k = 1..iters-1 computes S_{k+1} ----
        for k in range(1, iters):
            wt = wpool.tile([128, 256], DT)
            nc.sync.dma_start(wt[:].rearrange("p (w m) -> p w m", w=2),
                              wd_d[k - 1].rearrange("w p m -> p w m").bitcast(DT))
            cur, prev = (s_a, s_b) if k % 2 == 1 else (s_b, s_a)
            dest = prev
            pt = psum.tile([128, q], F32)
            for s in range(nsl):
                sl = slice(512 * s, 512 * (s + 1))
                nc.tensor.matmul(pt[:, sl], id_sb[:],
                                 b_sb[:, sl],
                                 start=True, stop=False)
                nc.tensor.matmul(pt[:, sl], wt[:, 0:128],
                                 cur[:, sl],
                                 start=False, stop=(k == 1))
                if k > 1:
                    nc.tensor.matmul(pt[:, sl], wt[:, 128:256],
                                     prev[:, sl],
                                     start=False, stop=True)
            for s in range(nsl):
                sl = slice(512 * s, 512 * (s + 1))
                if s % 2 == 0:
                    nc.vector.tensor_scalar_max(dest[:, sl], pt[:, sl], 0.0)
                else:
                    nc.scalar.activation(dest[:, sl], pt[:, sl],
                                         mybir.ActivationFunctionType.Relu)

        final = s_a if iters % 2 == 1 else s_b
        if iters == 1:
            final = s_a
        for g in range(4):
            for s in range(nsl):
                nc.sync.dma_start(
                    out_d[:, g * q + 512 * s: g * q + 512 * (s + 1)],
                    final[32 * g:32 * (g + 1), 512 * s:512 * (s + 1)].bitcast(F32),
                )

    nc.finalize()
    return nc


def host_prep(A: np.ndarray, iters: int):
    """Replicate the reference's fp32 scalar math and build device weights."""
    A = np.asarray(A, dtype=np.float32)
    AtA = (A.T @ A).astype(np.float32)
    L = np.linalg.svd(AtA, compute_uv=False)[0].astype(np.float32)
    step = (np.float32(1.0) / L).astype(np.float32)
    W = (np.eye(K, dtype=np.float32) - step * AtA).astype(np.float32)
    As = (step * A).astype(np.float32)

    # t/mu sequence in fp32 exactly like the reference scan
    t = np.float32(1.0)
    mus = []
    for _ in range(1, iters):
        t_new = (np.float32(0.5) * (np.float32(1.0) +
                 np.sqrt(np.float32(1.0) + np.float32(4.0) * t * t))).astype(np.float32)
        mus.append(((t - np.float32(1.0)) / t_new).astype(np.float32))
        t = t_new

    # folded per-iteration block-diagonal weights (lhsT = diag4(scaled W).T)
    Wt = W.T.astype(np.float64)
    wd = np.zeros((max(iters - 1, 1), 2, 128, 128), dtype=np.float32)
    for i, mu in enumerate(mus):
        wc = ((1.0 + np.float64(mu)) * Wt).astype(np.float32)
        wp = ((-np.float64(mu)) * Wt).astype(np.float32)
        for g in range(4):
            wd[i, 0, 32 * g:32 * (g + 1), 32 * g:32 * (g + 1)] = wc
            wd[i, 1, 32 * g:32 * (g + 1), 32 * g:32 * (g + 1)] = wp

    apad = np.zeros((4, M, 128), dtype=np.float32)
    for g in range(4):
        apad[g, :, 32 * g:32 * (g + 1)] = As
    ident = np.eye(128, dtype=np.float32)
    return wd, apad, ident


_PROGRAM_CACHE = {}


def _get_program(ns, iters):
    key = (ns, iters, str(MM_DTYPE))
    if key not in _PROGRAM_CACHE:
        _PROGRAM_CACHE[key] = build_program(ns, iters)
    return _PROGRAM_CACHE[key]


def kernel(X: np.ndarray, A: np.ndarray) -> np.ndarray:
    global LAST_RESULTS
    X = np.ascontiguousarray(np.asarray(X, dtype=np.float32))
    A = np.ascontiguousarray(np.asarray(A, dtype=np.float32))
    assert X.shape == (M, N_FULL) and A.shape == (M, K)

    ns = N_FULL // N_CORES
    wd, apad, ident = host_prep(A, ITERS)
    nc = _get_program(ns, ITERS)

    in_maps = []
    for c in range(N_CORES):
        in_maps.append({
            "x": np.ascontiguousarray(X[:, c * ns:(c + 1) * ns]),
            "apad": apad,
            "wd": wd,
            "ident": ident,
        })

    res = run_bass_kernel_spmd(nc, in_maps, core_ids=list(range(N_CORES)))
    LAST_RESULTS = res
    S = np.concatenate([res.results[c]["s_out"] for c in range(N_CORES)], axis=1)
    return np.ascontiguousarray(S.astype(np.float32))



# revision 5
# speedup vs baseline: 4.5049x; 4.5049x over previous
"""Trainium2 Bass kernel for nn_LsqNonneg: batched NNLS via heavy-ball projected gradient.

Math: the reference runs 200 FISTA iterations converging to the NNLS solution
S* (within ~3e-3 of it).  We converge to the same fixed point with a warm
start + constant-momentum heavy-ball iteration, which needs only ~40 steps:

    AtA = A.T A,  eigs: L = lam_max, mu = lam_min
    alpha = 4/(sqrt(L)+sqrt(mu))^2,  beta = ((sqrt(k)-1)/(sqrt(k)+1))^2, k=L/mu
    B   = alpha * A.T X                    [32, N]
    S0  = relu((1/L) A.T X) = relu(c0*B),  c0 = 1/(L*alpha)
    S1  = relu(W S0 + B),                  W  = I - alpha*AtA
    S_{k+1} = relu(Wc S_k - beta*S_{k-1} + B),  Wc = (1+beta)I - alpha*AtA

All iteration weights are constant -> loaded once, no per-iteration streaming.

Device layout (per core, NS=4096 columns): packed [128, 512] per slice s:
partition group g (rows 32g..32g+31) of slice s holds original columns
[g*1024 + 512*s, g*1024 + 512*s + 512).  Weights are diag4 [128,128] blocks so
one full-array matmul advances 4 column blocks; one slice = one PSUM bank.
Per step per slice: 3 accumulating matmuls (ident@B start, Wc@S_k,
(-beta I)@S_{k-1} stop) then relu psum->S (slice 0 on VectorE, slice 1 on
ScalarE) overlapped with the other slice's matmuls.
"""

import os
import sys

import numpy as np

for _p in ("/opt/trn_rl_repo", "/root/.axon_site/_ro/trn_rl_repo"):
    if os.path.isdir(_p) and _p not in sys.path:
        sys.path.append(_p)

from contextlib import ExitStack

import concourse.bass as bass
import concourse.bacc as bacc
import concourse.tile as tile
from concourse import mybir
from concourse.bass_utils import run_bass_kernel_spmd

M, K, N_FULL, N_CORES = 512, 32, 32768, 8
NSTEP = 40               # heavy-ball steps after the warm start

F32 = mybir.dt.float32
F32R = mybir.dt.float32r

MM_DTYPE = F32R

LAST_RESULTS = None  # BassKernelResults of the most recent run (for test.py)


def build_program(ns: int, nstep: int, c0: float, mm_dtype=MM_DTYPE):
    """Build the SPMD Bass program for one core holding `ns` columns."""
    DT = mm_dtype
    assert ns == 4096
    SL = 512             # columns per slice (one PSUM bank)
    NSL = 2              # slices

    nc = bacc.Bacc("TRN2", target_bir_lowering=False)

    x_d = nc.dram_tensor("x", [M, ns], F32, kind="ExternalInput")
    apad_d = nc.dram_tensor("apad", [4, M, 128], F32, kind="ExternalInput")
    idm_d = nc.dram_tensor("idm", [128, 128], F32, kind="ExternalInput")
    w1_d = nc.dram_tensor("w1", [128, 128], F32, kind="ExternalInput")
    wc_d = nc.dram_tensor("wc", [128, 128], F32, kind="ExternalInput")
    wp_d = nc.dram_tensor("wp", [128, 128], F32, kind="ExternalInput")
    out_d = nc.dram_tensor("s_out", [K, ns], F32, kind="ExternalOutput")

    with ExitStack() as ctx:
        tc = ctx.enter_context(tile.TileContext(nc))
        persist = ctx.enter_context(tc.tile_pool(name="persist", bufs=1))
        psum = ctx.enter_context(tc.tile_pool(name="psum", bufs=4, space="PSUM"))

        # --- small constant loads first ---
        apc = persist.tile([128, 16 * 128], DT)  # (g,c) chunk at 128*(4g+c)
        nc.sync.dma_start(
            apc[:].rearrange("p (g c m) -> p g c m", g=4, c=4),
            apad_d.rearrange("g (c p) m -> p g c m", c=4, p=128).bitcast(DT))
        id_sb = persist.tile([128, 128], DT)
        w1_sb = persist.tile([128, 128], DT)
        wc_sb = persist.tile([128, 128], DT)
        wp_sb = persist.tile([128, 128], DT)
        nc.sync.dma_start(id_sb[:], idm_d[:].bitcast(DT))
        nc.sync.dma_start(w1_sb[:], w1_d[:].bitcast(DT))
        nc.sync.dma_start(wc_sb[:], wc_d[:].bitcast(DT))
        nc.sync.dma_start(wp_sb[:], wp_d[:].bitcast(DT))

        # --- X staging: per row-chunk c, slice-major column layout ---
        # xt[c] cols: 2048*s + 512*g + j  <->  x_d col g*1024 + 512*s + j
        xts = [persist.tile([128, ns], DT, name=f"xt{c}") for c in range(4)]
        for s in range(NSL):
            for c in range(4):
                src = x_d[128 * c:128 * (c + 1), :].rearrange(
                    "p (g b j) -> p b g j", g=4, b=NSL, j=SL)[:, s]
                dst = xts[c][:, 2048 * s:2048 * (s + 1)].rearrange(
                    "p (g j) -> p g j", g=4)
                nc.sync.dma_start(dst, src.bitcast(DT))

        # --- B = alpha * A.T X  (packed layout), per slice ---
        pb = [psum.tile([128, SL], F32, tag=f"pb{s}", name=f"pb{s}", bufs=1) for s in range(NSL)]
        for s in range(NSL):
            for c in range(4):
                for g in range(4):
                    nc.tensor.matmul(
                        pb[s][:],
                        apc[:, 128 * (4 * g + c):128 * (4 * g + c + 1)],
                        xts[c][:, 2048 * s + SL * g: 2048 * s + SL * (g + 1)],
                        start=(c == 0 and g == 0),
                        stop=(c == 3 and g == 3),
                    )

        b_sb = [persist.tile([128, SL], DT, name=f"b_sb{s}") for s in range(NSL)]
        s_a = [persist.tile([128, SL], DT, name=f"s_a{s}") for s in range(NSL)]
        s_b = [persist.tile([128, SL], DT, name=f"s_b{s}") for s in range(NSL)]

        # drain B to SBUF and warm-start S0 = relu(c0 * B)
        nc.vector.tensor_copy(b_sb[0][:], pb[0][:])
        nc.scalar.copy(b_sb[1][:], pb[1][:])
        nc.vector.tensor_scalar(s_a[0][:], b_sb[0][:], c0, 0.0,
                                mybir.AluOpType.mult, mybir.AluOpType.max)
        nc.scalar.activation(s_a[1][:], b_sb[1][:],
                             mybir.ActivationFunctionType.Relu, scale=c0)

        # --- heavy-ball loop: step k computes S_{k+1} (S_k in cur) ---
        for k in range(1, nstep + 1):
            cur = s_a if k % 2 == 1 else s_b
            dest = s_b if k % 2 == 1 else s_a
            pts = []
            for s in range(NSL):
                pt = psum.tile([128, SL], F32, tag=f"pt{s}", name=f"pt{s}", bufs=3)
                nc.tensor.matmul(pt[:], id_sb[:], b_sb[s][:],
                                 start=True, stop=False)
                if k == 1:
                    nc.tensor.matmul(pt[:], w1_sb[:], cur[s][:],
                                     start=False, stop=True)
                else:
                    nc.tensor.matmul(pt[:], wc_sb[:], cur[s][:],
                                     start=False, stop=False)
                    nc.tensor.matmul(pt[:], wp_sb[:], dest[s][:],
                                     start=False, stop=True)
                pts.append(pt)
            nc.vector.tensor_scalar_max(dest[0][:], pts[0][:], 0.0)
            nc.scalar.activation(dest[1][:], pts[1][:],
                                 mybir.ActivationFunctionType.Relu)

        final = s_a if nstep % 2 == 0 else s_b
        for s in range(NSL):
            for g in range(4):
                nc.sync.dma_start(
                    out_d[:, g * 1024 + SL * s: g * 1024 + SL * (s + 1)],
                    final[s][K * g:K * (g + 1), :].bitcast(F32),
                )

    nc.finalize()
    return nc


def host_prep(A: np.ndarray, nstep: int):
    """Heavy-ball coefficients + constant device weights from A."""
    A = np.asarray(A, dtype=np.float32)
    AtA = (A.T @ A).astype(np.float64)
    ev = np.linalg.eigvalsh(AtA)
    L, mu = float(ev[-1]), float(ev[0])
    kap = L / mu
    alpha = 4.0 / (np.sqrt(L) + np.sqrt(mu)) ** 2
    beta = ((np.sqrt(kap) - 1.0) / (np.sqrt(kap) + 1.0)) ** 2
    c0 = (1.0 / L) / alpha

    W1 = (np.eye(K) - alpha * AtA)
    Wc = ((1.0 + beta) * np.eye(K) - alpha * AtA)

    def diag4(Wt):
        out = np.zeros((128, 128), dtype=np.float32)
        for g in range(4):
            out[K * g:K * (g + 1), K * g:K * (g + 1)] = Wt.astype(np.float32)
        return out

    w1 = diag4(W1.T)
    wc = diag4(Wc.T)
    wp = (-beta * np.eye(128)).astype(np.float32)
    idm = np.eye(128, dtype=np.float32)

    As = (alpha * A.astype(np.float64)).astype(np.float32)  # [M, K]
    apad = np.zeros((4, M, 128), dtype=np.float32)
    for g in range(4):
        apad[g, :, K * g:K * (g + 1)] = As
    return apad, idm, w1, wc, wp, float(c0)


_PROGRAM_CACHE = {}


def _get_program(ns, nstep, c0):
    key = (ns, nstep, round(c0, 10), str(MM_DTYPE))
    if key not in _PROGRAM_CACHE:
        _PROGRAM_CACHE[key] = build_program(ns, nstep, c0)
    return _PROGRAM_CACHE[key]


def kernel(X: np.ndarray, A: np.ndarray) -> np.ndarray:
    global LAST_RESULTS
    X = np.ascontiguousarray(np.asarray(X, dtype=np.float32))
    A = np.ascontiguousarray(np.asarray(A, dtype=np.float32))
    assert X.shape == (M, N_FULL) and A.shape == (M, K)

    ns = N_FULL // N_CORES
    apad, idm, w1, wc, wp, c0 = host_prep(A, NSTEP)
    nc = _get_program(ns, NSTEP, c0)

    in_maps = []
    for c in range(N_CORES):
        in_maps.append({
            "x": np.ascontiguousarray(X[:, c * ns:(c + 1) * ns]),
            "apad": apad,
            "idm": idm,
            "w1": w1,
            "wc": wc,
            "wp": wp,
        })

    res = run_bass_kernel_spmd(nc, in_maps, core_ids=list(range(N_CORES)))
    LAST_RESULTS = res
    S = np.concatenate([res.results[c]["s_out"] for c in range(N_CORES)], axis=1)
    return np.ascontiguousarray(S.astype(np.float32))


# revision 7
# speedup vs baseline: 4.7892x; 1.0631x over previous
"""Trainium2 Bass kernel for nn_LsqNonneg: batched NNLS via heavy-ball projected gradient.

Math: the reference runs 200 FISTA iterations converging to the NNLS solution
S* (within ~3e-3 of it).  We converge to the same fixed point with a warm
start + constant-momentum heavy-ball iteration, which needs only ~40 steps:

    AtA = A.T A,  eigs: L = lam_max, mu = lam_min
    alpha = 4/(sqrt(L)+sqrt(mu))^2,  beta = ((sqrt(k)-1)/(sqrt(k)+1))^2, k=L/mu
    B   = alpha * A.T X                    [32, N]
    S0  = relu((1/L) A.T X) = relu(c0*B),  c0 = 1/(L*alpha)
    S1  = relu(W S0 + B),                  W  = I - alpha*AtA
    S_{k+1} = relu(Wc S_k - beta*S_{k-1} + B),  Wc = (1+beta)I - alpha*AtA

All iteration weights are constant -> loaded once, no per-iteration streaming.

Device layout (per core, NS=4096 columns): packed [128, 512] per slice s:
partition group g (rows 32g..32g+31) of slice s holds original columns
[g*1024 + 512*s, g*1024 + 512*s + 512).  Weights are diag4 [128,128] blocks so
one full-array matmul advances 4 column blocks; one slice = one PSUM bank.
Per step per slice: 3 accumulating matmuls (ident@B start, Wc@S_k,
(-beta I)@S_{k-1} stop) then relu psum->S (slice 0 on VectorE, slice 1 on
ScalarE) overlapped with the other slice's matmuls.
"""

import os
import sys

import numpy as np

for _p in ("/opt/trn_rl_repo", "/root/.axon_site/_ro/trn_rl_repo"):
    if os.path.isdir(_p) and _p not in sys.path:
        sys.path.append(_p)

from contextlib import ExitStack

import concourse.bass as bass
import concourse.bacc as bacc
import concourse.tile as tile
from concourse import mybir
from concourse.bass_utils import run_bass_kernel_spmd

M, K, N_FULL, N_CORES = 512, 32, 32768, 8
NSTEP = 40               # heavy-ball steps after the warm start

F32 = mybir.dt.float32
F32R = mybir.dt.float32r
F16 = mybir.dt.float16

MM_DTYPE = F32R

LAST_RESULTS = None  # BassKernelResults of the most recent run (for test.py)


def build_program(ns: int, nstep: int, c0: float, alpha: float, mm_dtype=MM_DTYPE):
    """Build the SPMD Bass program for one core holding `ns` columns."""
    DT = mm_dtype
    assert ns == 4096
    SL = 512             # columns per slice (one PSUM bank)
    NSL = 2              # slices

    nc = bacc.Bacc("TRN2", target_bir_lowering=False)

    x_d = nc.dram_tensor("x", [M, ns], F16, kind="ExternalInput")
    apad_d = nc.dram_tensor("apad", [4, M, 128], F16, kind="ExternalInput")
    idm_d = nc.dram_tensor("idm", [128, 128], F32, kind="ExternalInput")
    w1_d = nc.dram_tensor("w1", [128, 128], F32, kind="ExternalInput")
    wc_d = nc.dram_tensor("wc", [128, 128], F32, kind="ExternalInput")
    wp_d = nc.dram_tensor("wp", [128, 128], F32, kind="ExternalInput")
    out_d = nc.dram_tensor("s_out", [K, ns], F32, kind="ExternalOutput")

    with ExitStack() as ctx:
        tc = ctx.enter_context(tile.TileContext(nc))
        persist = ctx.enter_context(tc.tile_pool(name="persist", bufs=1))
        psum = ctx.enter_context(tc.tile_pool(name="psum", bufs=4, space="PSUM"))

        # --- X staging first (critical path), slice-major column layout ---
        # xt[c] cols: 2048*s + 512*g + j  <->  x_d col g*1024 + 512*s + j
        xts = [persist.tile([128, ns], F16, name=f"xt{c}") for c in range(4)]
        for s in range(NSL):
            for c in range(4):
                xsrc = x_d[128 * c:128 * (c + 1), :].rearrange(
                    "p (g b j) -> p b g j", g=4, b=NSL, j=SL)[:, s]
                dst = xts[c][:, 2048 * s:2048 * (s + 1)].rearrange(
                    "p (g j) -> p g j", g=4)
                nc.sync.dma_start(dst, xsrc)

        # --- constants on the scalar HWDGE ring (parallel with X) ---
        apc = persist.tile([128, 16 * 128], F16)  # (g,c) chunk at 128*(4g+c)
        nc.scalar.dma_start(
            apc[:].rearrange("p (g c m) -> p g c m", g=4, c=4),
            apad_d.rearrange("g (c p) m -> p g c m", c=4, p=128))
        id_sb = persist.tile([128, 128], DT)
        w1_sb = persist.tile([128, 128], DT)
        wc_sb = persist.tile([128, 128], DT)
        wp_sb = persist.tile([128, 128], DT)
        nc.scalar.dma_start(id_sb[:], idm_d[:].bitcast(DT))
        nc.scalar.dma_start(w1_sb[:], w1_d[:].bitcast(DT))
        nc.scalar.dma_start(wc_sb[:], wc_d[:].bitcast(DT))
        nc.scalar.dma_start(wp_sb[:], wp_d[:].bitcast(DT))

        # --- B = alpha * A.T X  (packed layout), per slice ---
        pb = [psum.tile([128, SL], F32, tag=f"pb{s}", name=f"pb{s}", bufs=1) for s in range(NSL)]
        for s in range(NSL):
            for c in range(4):
                for g in range(4):
                    nc.tensor.matmul(
                        pb[s][:],
                        apc[:, 128 * (4 * g + c):128 * (4 * g + c + 1)],
                        xts[c][:, 2048 * s + SL * g: 2048 * s + SL * (g + 1)],
                        start=(c == 0 and g == 0),
                        stop=(c == 3 and g == 3),
                    )

        b_sb = [persist.tile([128, SL], DT, name=f"b_sb{s}") for s in range(NSL)]
        s_a = [persist.tile([128, SL], DT, name=f"s_a{s}") for s in range(NSL)]
        s_b = [persist.tile([128, SL], DT, name=f"s_b{s}") for s in range(NSL)]

        # drain B = alpha * (A.T X psum) to SBUF, warm-start S0 = relu(c0 * B)
        nc.vector.tensor_scalar_mul(b_sb[0][:], pb[0][:], alpha)
        nc.scalar.activation(b_sb[1][:], pb[1][:],
                             mybir.ActivationFunctionType.Copy, scale=alpha)
        nc.vector.tensor_scalar(s_a[0][:], b_sb[0][:], c0, 0.0,
                                mybir.AluOpType.mult, mybir.AluOpType.max)
        nc.scalar.activation(s_a[1][:], b_sb[1][:],
                             mybir.ActivationFunctionType.Relu, scale=c0)

        # --- heavy-ball loop: step k computes S_{k+1} (S_k in cur) ---
        for k in range(1, nstep + 1):
            cur = s_a if k % 2 == 1 else s_b
            dest = s_b if k % 2 == 1 else s_a
            pts = []
            for s in range(NSL):
                pt = psum.tile([128, SL], F32, tag=f"pt{s}", name=f"pt{s}", bufs=3)
                nc.tensor.matmul(pt[:], id_sb[:], b_sb[s][:],
                                 start=True, stop=False)
                if k == 1:
                    nc.tensor.matmul(pt[:], w1_sb[:], cur[s][:],
                                     start=False, stop=True)
                else:
                    nc.tensor.matmul(pt[:], wc_sb[:], cur[s][:],
                                     start=False, stop=False)
                    nc.tensor.matmul(pt[:], wp_sb[:], dest[s][:],
                                     start=False, stop=True)
                pts.append(pt)
            H = SL // 2
            nc.vector.tensor_scalar_max(dest[0][:, 0:H], pts[0][:, 0:H], 0.0)
            nc.scalar.activation(dest[0][:, H:SL], pts[0][:, H:SL],
                                 mybir.ActivationFunctionType.Relu)
            nc.vector.tensor_scalar_max(dest[1][:, 0:H], pts[1][:, 0:H], 0.0)
            nc.scalar.activation(dest[1][:, H:SL], pts[1][:, H:SL],
                                 mybir.ActivationFunctionType.Relu)

        final = s_a if nstep % 2 == 0 else s_b
        for s in range(NSL):
            for g in range(4):
                nc.sync.dma_start(
                    out_d[:, g * 1024 + SL * s: g * 1024 + SL * (s + 1)],
                    final[s][K * g:K * (g + 1), :].bitcast(F32),
                )

    nc.finalize()
    return nc


def host_prep(A: np.ndarray, nstep: int):
    """Heavy-ball coefficients + constant device weights from A."""
    A = np.asarray(A, dtype=np.float32)
    AtA = (A.T @ A).astype(np.float64)
    ev = np.linalg.eigvalsh(AtA)
    L, mu = float(ev[-1]), float(ev[0])
    kap = L / mu
    alpha = 4.0 / (np.sqrt(L) + np.sqrt(mu)) ** 2
    beta = ((np.sqrt(kap) - 1.0) / (np.sqrt(kap) + 1.0)) ** 2
    c0 = (1.0 / L) / alpha

    W1 = (np.eye(K) - alpha * AtA)
    Wc = ((1.0 + beta) * np.eye(K) - alpha * AtA)

    def diag4(Wt):
        out = np.zeros((128, 128), dtype=np.float32)
        for g in range(4):
            out[K * g:K * (g + 1), K * g:K * (g + 1)] = Wt.astype(np.float32)
        return out

    w1 = diag4(W1.T)
    wc = diag4(Wc.T)
    wp = (-beta * np.eye(128)).astype(np.float32)
    idm = np.eye(128, dtype=np.float32)

    apad = np.zeros((4, M, 128), dtype=np.float16)
    for g in range(4):
        apad[g, :, K * g:K * (g + 1)] = A.astype(np.float16)
    return apad, idm, w1, wc, wp, float(c0), float(alpha)


_PROGRAM_CACHE = {}


def _get_program(ns, nstep, c0, alpha):
    key = (ns, nstep, round(c0, 10), round(alpha, 12), str(MM_DTYPE))
    if key not in _PROGRAM_CACHE:
        _PROGRAM_CACHE[key] = build_program(ns, nstep, c0, alpha)
    return _PROGRAM_CACHE[key]


def kernel(X: np.ndarray, A: np.ndarray) -> np.ndarray:
    global LAST_RESULTS
    X = np.ascontiguousarray(np.asarray(X, dtype=np.float32))
    A = np.ascontiguousarray(np.asarray(A, dtype=np.float32))
    assert X.shape == (M, N_FULL) and A.shape == (M, K)

    ns = N_FULL // N_CORES
    apad, idm, w1, wc, wp, c0, alpha = host_prep(A, NSTEP)
    nc = _get_program(ns, NSTEP, c0, alpha)

    in_maps = []
    for c in range(N_CORES):
        in_maps.append({
            "x": np.ascontiguousarray(X[:, c * ns:(c + 1) * ns].astype(np.float16)),
            "apad": apad,
            "idm": idm,
            "w1": w1,
            "wc": wc,
            "wp": wp,
        })

    res = run_bass_kernel_spmd(nc, in_maps, core_ids=list(range(N_CORES)))
    LAST_RESULTS = res
    S = np.concatenate([res.results[c]["s_out"] for c in range(N_CORES)], axis=1)
    return np.ascontiguousarray(S.astype(np.float32))


# revision 8
# speedup vs baseline: 5.2834x; 1.1032x over previous
"""Trainium2 Bass kernel for nn_LsqNonneg: batched NNLS via heavy-ball projected gradient.

Math: the reference runs 200 FISTA iterations converging to the NNLS solution
S* (within ~3e-3 of it).  We converge to the same fixed point with a warm
start + constant-momentum heavy-ball iteration, which needs only ~40 steps:

    AtA = A.T A,  eigs: L = lam_max, mu = lam_min
    alpha = 4/(sqrt(L)+sqrt(mu))^2,  beta = ((sqrt(k)-1)/(sqrt(k)+1))^2, k=L/mu
    B   = alpha * A.T X                    [32, N]
    S0  = relu((1/L) A.T X) = relu(c0*B),  c0 = 1/(L*alpha)
    S1  = relu(W S0 + B),                  W  = I - alpha*AtA
    S_{k+1} = relu(Wc S_k - beta*S_{k-1} + B),  Wc = (1+beta)I - alpha*AtA

All iteration weights are constant -> loaded once, no per-iteration streaming.

Device layout (per core, NS=4096 columns): packed [128, 512] per slice s:
partition group g (rows 32g..32g+31) of slice s holds original columns
[g*1024 + 512*s, g*1024 + 512*s + 512).  Weights are diag4 [128,128] blocks so
one full-array matmul advances 4 column blocks; one slice = one PSUM bank.
Per step per slice: 3 accumulating matmuls (ident@B start, Wc@S_k,
(-beta I)@S_{k-1} stop) then relu psum->S (slice 0 on VectorE, slice 1 on
ScalarE) overlapped with the other slice's matmuls.
"""

import os
import sys

import numpy as np

for _p in ("/opt/trn_rl_repo", "/root/.axon_site/_ro/trn_rl_repo"):
    if os.path.isdir(_p) and _p not in sys.path:
        sys.path.append(_p)

from contextlib import ExitStack

import concourse.bass as bass
import concourse.bacc as bacc
import concourse.tile as tile
from concourse import mybir
from concourse.bass_utils import run_bass_kernel_spmd

M, K, N_FULL, N_CORES = 512, 32, 32768, 8
NSTEP = 40               # heavy-ball steps after the warm start

F32 = mybir.dt.float32
F32R = mybir.dt.float32r
F16 = mybir.dt.float16

MM_DTYPE = F32R

LAST_RESULTS = None  # BassKernelResults of the most recent run (for test.py)


def build_program(ns: int, nstep: int, c0: float, alpha: float, mm_dtype=MM_DTYPE):
    """Build the SPMD Bass program for one core holding `ns` columns."""
    DT = mm_dtype
    assert ns == 4096
    SL = 512             # columns per slice (one PSUM bank)
    NSL = 2              # slices

    nc = bacc.Bacc("TRN2", target_bir_lowering=False)

    x_d = nc.dram_tensor("x", [M, ns], F16, kind="ExternalInput")
    apad_d = nc.dram_tensor("apad", [128, 16 * 128], F16, kind="ExternalInput")
    idm_d = nc.dram_tensor("idm", [128, 128], F32, kind="ExternalInput")
    w1_d = nc.dram_tensor("w1", [128, 128], F32, kind="ExternalInput")
    wc_d = nc.dram_tensor("wc", [128, 128], F32, kind="ExternalInput")
    wp_d = nc.dram_tensor("wp", [128, 128], F32, kind="ExternalInput")
    out_d = nc.dram_tensor("s_out", [K, ns], F32, kind="ExternalOutput")

    with ExitStack() as ctx:
        tc = ctx.enter_context(tile.TileContext(nc))
        persist = ctx.enter_context(tc.tile_pool(name="persist", bufs=1))
        psum = ctx.enter_context(tc.tile_pool(name="psum", bufs=4, space="PSUM"))

        # --- X staging first (critical path), slice-major column layout ---
        # xt[c] cols: 2048*s + 512*g + j  <->  x_d col g*1024 + 512*s + j
        xts = [persist.tile([128, ns], F16, name=f"xt{c}") for c in range(4)]
        for s in range(NSL):
            for c in range(4):
                xsrc = x_d[128 * c:128 * (c + 1), :].rearrange(
                    "p (g b j) -> p b g j", g=4, b=NSL, j=SL)[:, s]
                dst = xts[c][:, 2048 * s:2048 * (s + 1)].rearrange(
                    "p (g j) -> p g j", g=4)
                nc.sync.dma_start(dst, xsrc)

        # --- constants on the scalar HWDGE ring (parallel with X) ---
        apc = persist.tile([128, 16 * 128], F16)  # (g,c) chunk at 128*(4g+c)
        nc.scalar.dma_start(apc[:], apad_d[:])
        id_sb = persist.tile([128, 128], DT)
        w1_sb = persist.tile([128, 128], DT)
        wc_sb = persist.tile([128, 128], DT)
        wp_sb = persist.tile([128, 128], DT)
        nc.scalar.dma_start(id_sb[:], idm_d[:].bitcast(DT))
        nc.scalar.dma_start(w1_sb[:], w1_d[:].bitcast(DT))
        nc.scalar.dma_start(wc_sb[:], wc_d[:].bitcast(DT))
        nc.scalar.dma_start(wp_sb[:], wp_d[:].bitcast(DT))

        # --- B = alpha * A.T X  (packed layout), per slice ---
        pb = [psum.tile([128, SL], F32, tag=f"pb{s}", name=f"pb{s}", bufs=1) for s in range(NSL)]
        for s in range(NSL):
            for c in range(4):
                for g in range(4):
                    nc.tensor.matmul(
                        pb[s][:],
                        apc[:, 128 * (4 * g + c):128 * (4 * g + c + 1)],
                        xts[c][:, 2048 * s + SL * g: 2048 * s + SL * (g + 1)],
                        start=(c == 0 and g == 0),
                        stop=(c == 3 and g == 3),
                    )

        b_sb = [persist.tile([128, SL], DT, name=f"b_sb{s}") for s in range(NSL)]
        s_a = [persist.tile([128, SL], DT, name=f"s_a{s}") for s in range(NSL)]
        s_b = [persist.tile([128, SL], DT, name=f"s_b{s}") for s in range(NSL)]

        # drain B = alpha * (A.T X psum) to SBUF, warm-start S0 = relu(c0 * B)
        nc.vector.tensor_scalar_mul(b_sb[0][:], pb[0][:], alpha)
        nc.scalar.activation(b_sb[1][:], pb[1][:],
                             mybir.ActivationFunctionType.Copy, scale=alpha)
        nc.vector.tensor_scalar(s_a[0][:], b_sb[0][:], c0, 0.0,
                                mybir.AluOpType.mult, mybir.AluOpType.max)
        nc.scalar.activation(s_a[1][:], b_sb[1][:],
                             mybir.ActivationFunctionType.Relu, scale=c0)

        # --- heavy-ball loop: step k computes S_{k+1} (S_k in cur) ---
        for k in range(1, nstep + 1):
            cur = s_a if k % 2 == 1 else s_b
            dest = s_b if k % 2 == 1 else s_a
            pts = []
            for s in range(NSL):
                pt = psum.tile([128, SL], F32, tag=f"pt{s}", name=f"pt{s}", bufs=3)
                nc.tensor.matmul(pt[:], id_sb[:], b_sb[s][:],
                                 start=True, stop=False)
                if k == 1:
                    nc.tensor.matmul(pt[:], w1_sb[:], cur[s][:],
                                     start=False, stop=True)
                else:
                    nc.tensor.matmul(pt[:], wc_sb[:], cur[s][:],
                                     start=False, stop=False)
                    nc.tensor.matmul(pt[:], wp_sb[:], dest[s][:],
                                     start=False, stop=True)
                pts.append(pt)
            nc.vector.tensor_scalar_max(dest[0][:], pts[0][:], 0.0)
            nc.scalar.activation(dest[1][:], pts[1][:],
                                 mybir.ActivationFunctionType.Relu)

        final = s_a if nstep % 2 == 0 else s_b
        for s in range(NSL):
            for g in range(4):
                nc.sync.dma_start(
                    out_d[:, g * 1024 + SL * s: g * 1024 + SL * (s + 1)],
                    final[s][K * g:K * (g + 1), :].bitcast(F32),
                )

    nc.finalize()
    return nc


def host_prep(A: np.ndarray, nstep: int):
    """Heavy-ball coefficients + constant device weights from A."""
    A = np.asarray(A, dtype=np.float32)
    AtA = (A.T @ A).astype(np.float64)
    ev = np.linalg.eigvalsh(AtA)
    L, mu = float(ev[-1]), float(ev[0])
    kap = L / mu
    alpha = 4.0 / (np.sqrt(L) + np.sqrt(mu)) ** 2
    beta = ((np.sqrt(kap) - 1.0) / (np.sqrt(kap) + 1.0)) ** 2
    c0 = (1.0 / L) / alpha

    W1 = (np.eye(K) - alpha * AtA)
    Wc = ((1.0 + beta) * np.eye(K) - alpha * AtA)

    def diag4(Wt):
        out = np.zeros((128, 128), dtype=np.float32)
        for g in range(4):
            out[K * g:K * (g + 1), K * g:K * (g + 1)] = Wt.astype(np.float32)
        return out

    w1 = diag4(W1.T)
    wc = diag4(Wc.T)
    wp = (-beta * np.eye(128)).astype(np.float32)
    idm = np.eye(128, dtype=np.float32)

    A16 = A.astype(np.float16)
    apad = np.zeros((128, 16 * 128), dtype=np.float16)
    for g in range(4):
        for c in range(4):
            blk = np.zeros((128, 128), dtype=np.float16)
            blk[:, K * g:K * (g + 1)] = A16[128 * c:128 * (c + 1), :]
            apad[:, 128 * (4 * g + c):128 * (4 * g + c + 1)] = blk
    return apad, idm, w1, wc, wp, float(c0), float(alpha)


_PROGRAM_CACHE = {}


def _get_program(ns, nstep, c0, alpha):
    key = (ns, nstep, round(c0, 10), round(alpha, 12), str(MM_DTYPE))
    if key not in _PROGRAM_CACHE:
        _PROGRAM_CACHE[key] = build_program(ns, nstep, c0, alpha)
    return _PROGRAM_CACHE[key]


def kernel(X: np.ndarray, A: np.ndarray) -> np.ndarray:
    global LAST_RESULTS
    X = np.ascontiguousarray(np.asarray(X, dtype=np.float32))
    A = np.ascontiguousarray(np.asarray(A, dtype=np.float32))
    assert X.shape == (M, N_FULL) and A.shape == (M, K)

    ns = N_FULL // N_CORES
    apad, idm, w1, wc, wp, c0, alpha = host_prep(A, NSTEP)
    nc = _get_program(ns, NSTEP, c0, alpha)

    in_maps = []
    for c in range(N_CORES):
        in_maps.append({
            "x": np.ascontiguousarray(X[:, c * ns:(c + 1) * ns].astype(np.float16)),
            "apad": apad,
            "idm": idm,
            "w1": w1,
            "wc": wc,
            "wp": wp,
        })

    res = run_bass_kernel_spmd(nc, in_maps, core_ids=list(range(N_CORES)))
    LAST_RESULTS = res
    S = np.concatenate([res.results[c]["s_out"] for c in range(N_CORES)], axis=1)
    return np.ascontiguousarray(S.astype(np.float32))


# revision 9
# speedup vs baseline: 5.9685x; 1.1297x over previous
"""Trainium2 Bass kernel for nn_LsqNonneg: batched NNLS via heavy-ball projected gradient.

Math: the reference runs 200 FISTA iterations converging to the NNLS solution
S* (within ~3e-3 of it).  We converge to the same fixed point with a warm
start + constant-momentum heavy-ball iteration, which needs only ~40 steps:

    AtA = A.T A,  eigs: L = lam_max, mu = lam_min
    alpha = 4/(sqrt(L)+sqrt(mu))^2,  beta = ((sqrt(k)-1)/(sqrt(k)+1))^2, k=L/mu
    B   = alpha * A.T X                    [32, N]
    S0  = relu((1/L) A.T X) = relu(c0*B),  c0 = 1/(L*alpha)
    S1  = relu(W S0 + B),                  W  = I - alpha*AtA
    S_{k+1} = relu(Wc S_k - beta*S_{k-1} + B),  Wc = (1+beta)I - alpha*AtA

All iteration weights are constant -> loaded once, no per-iteration streaming.

Device layout (per core, NS=4096 columns): packed [128, 512] per slice s:
partition group g (rows 32g..32g+31) of slice s holds original columns
[g*1024 + 512*s, g*1024 + 512*s + 512).  Weights are diag4 [128,128] blocks so
one full-array matmul advances 4 column blocks; one slice = one PSUM bank.
Per step per slice: 3 accumulating matmuls (ident@B start, Wc@S_k,
(-beta I)@S_{k-1} stop) then relu psum->S (slice 0 on VectorE, slice 1 on
ScalarE) overlapped with the other slice's matmuls.
"""

import os
import sys

import numpy as np

for _p in ("/opt/trn_rl_repo", "/root/.axon_site/_ro/trn_rl_repo"):
    if os.path.isdir(_p) and _p not in sys.path:
        sys.path.append(_p)

from contextlib import ExitStack

import concourse.bass as bass
import concourse.bacc as bacc
import concourse.tile as tile
from concourse import mybir
from concourse.bass_utils import run_bass_kernel_spmd

M, K, N_FULL, N_CORES = 512, 32, 32768, 8
NSTEP = 36               # heavy-ball steps after the warm start

F32 = mybir.dt.float32
F32R = mybir.dt.float32r
F16 = mybir.dt.float16

MM_DTYPE = F32R

LAST_RESULTS = None  # BassKernelResults of the most recent run (for test.py)


def build_program(ns: int, nstep: int, c0: float, alpha: float, mm_dtype=MM_DTYPE):
    """Build the SPMD Bass program for one core holding `ns` columns."""
    DT = mm_dtype
    assert ns == 4096
    SL = 512             # columns per slice (one PSUM bank)
    NSL = 2              # slices

    nc = bacc.Bacc("TRN2", target_bir_lowering=False)

    x_d = nc.dram_tensor("x", [M, ns], F16, kind="ExternalInput")
    apad_d = nc.dram_tensor("apad", [128, 16 * 128], F16, kind="ExternalInput")
    idm_d = nc.dram_tensor("idm", [128, 128], F32, kind="ExternalInput")
    w1_d = nc.dram_tensor("w1", [128, 128], F32, kind="ExternalInput")
    wc_d = nc.dram_tensor("wc", [128, 128], F32, kind="ExternalInput")
    wp_d = nc.dram_tensor("wp", [128, 128], F32, kind="ExternalInput")
    out_d = nc.dram_tensor("s_out", [K, ns], F32, kind="ExternalOutput")

    with ExitStack() as ctx:
        tc = ctx.enter_context(tile.TileContext(nc))
        persist = ctx.enter_context(tc.tile_pool(name="persist", bufs=1))
        psum = ctx.enter_context(tc.tile_pool(name="psum", bufs=4, space="PSUM"))

        # --- X staging first (critical path), slice-major column layout ---
        # xt[c] cols: 2048*s + 512*g + j  <->  x_d col g*1024 + 512*s + j
        xts = [persist.tile([128, ns], F16, name=f"xt{c}") for c in range(4)]
        for s in range(NSL):
            for c in range(4):
                xsrc = x_d[128 * c:128 * (c + 1), :].rearrange(
                    "p (g b j) -> p b g j", g=4, b=NSL, j=SL)[:, s]
                dst = xts[c][:, 2048 * s:2048 * (s + 1)].rearrange(
                    "p (g j) -> p g j", g=4)
                nc.sync.dma_start(dst, xsrc)

        # --- constants on the scalar HWDGE ring (parallel with X) ---
        apc = persist.tile([128, 16 * 128], F16)  # (g,c) chunk at 128*(4g+c)
        nc.scalar.dma_start(apc[:], apad_d[:])
        id_sb = persist.tile([128, 128], DT)
        w1_sb = persist.tile([128, 128], DT)
        wc_sb = persist.tile([128, 128], DT)
        wp_sb = persist.tile([128, 128], DT)
        nc.scalar.dma_start(id_sb[:], idm_d[:].bitcast(DT))
        nc.scalar.dma_start(w1_sb[:], w1_d[:].bitcast(DT))
        nc.scalar.dma_start(wc_sb[:], wc_d[:].bitcast(DT))
        nc.scalar.dma_start(wp_sb[:], wp_d[:].bitcast(DT))

        # --- B = alpha * A.T X  (packed layout), per slice ---
        pb = [psum.tile([128, SL], F32, tag=f"pb{s}", name=f"pb{s}", bufs=1) for s in range(NSL)]
        for c in range(4):
            for s in range(NSL):
                for g in range(4):
                    nc.tensor.matmul(
                        pb[s][:],
                        apc[:, 128 * (4 * g + c):128 * (4 * g + c + 1)],
                        xts[c][:, 2048 * s + SL * g: 2048 * s + SL * (g + 1)],
                        start=(c == 0 and g == 0),
                        stop=(c == 3 and g == 3),
                    )

        b_sb = [persist.tile([128, SL], DT, name=f"b_sb{s}") for s in range(NSL)]
        s_a = [persist.tile([128, SL], DT, name=f"s_a{s}") for s in range(NSL)]
        s_b = [persist.tile([128, SL], DT, name=f"s_b{s}") for s in range(NSL)]

        # drain B = alpha * (A.T X psum) to SBUF, warm-start S0 = relu(c0 * B)
        nc.vector.tensor_scalar_mul(b_sb[0][:], pb[0][:], alpha)
        nc.scalar.activation(b_sb[1][:], pb[1][:],
                             mybir.ActivationFunctionType.Copy, scale=alpha)
        nc.vector.tensor_scalar(s_a[0][:], b_sb[0][:], c0, 0.0,
                                mybir.AluOpType.mult, mybir.AluOpType.max)
        nc.scalar.activation(s_a[1][:], b_sb[1][:],
                             mybir.ActivationFunctionType.Relu, scale=c0)

        # --- heavy-ball loop: step k computes S_{k+1} (S_k in cur) ---
        for k in range(1, nstep + 1):
            cur = s_a if k % 2 == 1 else s_b
            dest = s_b if k % 2 == 1 else s_a
            pts = []
            for s in range(NSL):
                pt = psum.tile([128, SL], F32, tag=f"pt{s}", name=f"pt{s}", bufs=3)
                nc.tensor.matmul(pt[:], id_sb[:], b_sb[s][:],
                                 start=True, stop=False)
                if k == 1:
                    nc.tensor.matmul(pt[:], w1_sb[:], cur[s][:],
                                     start=False, stop=True)
                else:
                    nc.tensor.matmul(pt[:], wp_sb[:], dest[s][:],
                                     start=False, stop=False)
                    nc.tensor.matmul(pt[:], wc_sb[:], cur[s][:],
                                     start=False, stop=True)
                pts.append(pt)
            nc.vector.tensor_scalar_max(dest[0][:], pts[0][:], 0.0)
            nc.scalar.activation(dest[1][:], pts[1][:],
                                 mybir.ActivationFunctionType.Relu)

        final = s_a if nstep % 2 == 0 else s_b
        for s in range(NSL):
            for g in range(4):
                eng = nc.sync if g % 2 == 0 else nc.scalar
                eng.dma_start(
                    out_d[:, g * 1024 + SL * s: g * 1024 + SL * (s + 1)],
                    final[s][K * g:K * (g + 1), :].bitcast(F32),
                )

    nc.finalize()
    return nc


def host_prep(A: np.ndarray, nstep: int):
    """Heavy-ball coefficients + constant device weights from A."""
    A = np.asarray(A, dtype=np.float32)
    AtA = (A.T @ A).astype(np.float64)
    ev = np.linalg.eigvalsh(AtA)
    L, mu = float(ev[-1]), float(ev[0])
    kap = L / mu
    alpha = 4.0 / (np.sqrt(L) + np.sqrt(mu)) ** 2
    beta = ((np.sqrt(kap) - 1.0) / (np.sqrt(kap) + 1.0)) ** 2
    c0 = (1.0 / L) / alpha

    W1 = (np.eye(K) - alpha * AtA)
    Wc = ((1.0 + beta) * np.eye(K) - alpha * AtA)

    def diag4(Wt):
        out = np.zeros((128, 128), dtype=np.float32)
        for g in range(4):
            out[K * g:K * (g + 1), K * g:K * (g + 1)] = Wt.astype(np.float32)
        return out

    w1 = diag4(W1.T)
    wc = diag4(Wc.T)
    wp = (-beta * np.eye(128)).astype(np.float32)
    idm = np.eye(128, dtype=np.float32)

    A16 = A.astype(np.float16)
    apad = np.zeros((128, 16 * 128), dtype=np.float16)
    for g in range(4):
        for c in range(4):
            blk = np.zeros((128, 128), dtype=np.float16)
            blk[:, K * g:K * (g + 1)] = A16[128 * c:128 * (c + 1), :]
            apad[:, 128 * (4 * g + c):128 * (4 * g + c + 1)] = blk
    return apad, idm, w1, wc, wp, float(c0), float(alpha)


_PROGRAM_CACHE = {}


def _get_program(ns, nstep, c0, alpha):
    key = (ns, nstep, round(c0, 10), round(alpha, 12), str(MM_DTYPE))
    if key not in _PROGRAM_CACHE:
        _PROGRAM_CACHE[key] = build_program(ns, nstep, c0, alpha)
    return _PROGRAM_CACHE[key]


def kernel(X: np.ndarray, A: np.ndarray) -> np.ndarray:
    global LAST_RESULTS
    X = np.ascontiguousarray(np.asarray(X, dtype=np.float32))
    A = np.ascontiguousarray(np.asarray(A, dtype=np.float32))
    assert X.shape == (M, N_FULL) and A.shape == (M, K)

    ns = N_FULL // N_CORES
    apad, idm, w1, wc, wp, c0, alpha = host_prep(A, NSTEP)
    nc = _get_program(ns, NSTEP, c0, alpha)

    in_maps = []
    for c in range(N_CORES):
        in_maps.append({
            "x": np.ascontiguousarray(X[:, c * ns:(c + 1) * ns].astype(np.float16)),
            "apad": apad,
            "idm": idm,
            "w1": w1,
            "wc": wc,
            "wp": wp,
        })

    res = run_bass_kernel_spmd(nc, in_maps, core_ids=list(range(N_CORES)))
    LAST_RESULTS = res
    S = np.concatenate([res.results[c]["s_out"] for c in range(N_CORES)], axis=1)
    return np.ascontiguousarray(S.astype(np.float32))


# revision 10
# speedup vs baseline: 6.6036x; 1.1064x over previous
"""Trainium2 Bass kernel for nn_LsqNonneg: batched NNLS via heavy-ball projected gradient.

Math: the reference runs 200 FISTA iterations converging to the NNLS solution
S* (within ~3e-3 of it).  We converge to the same fixed point with a warm
start + constant-momentum heavy-ball iteration, which needs only ~40 steps:

    AtA = A.T A,  eigs: L = lam_max, mu = lam_min
    alpha = 4/(sqrt(L)+sqrt(mu))^2,  beta = ((sqrt(k)-1)/(sqrt(k)+1))^2, k=L/mu
    B   = alpha * A.T X                    [32, N]
    S0  = relu((1/L) A.T X) = relu(c0*B),  c0 = 1/(L*alpha)
    S1  = relu(W S0 + B),                  W  = I - alpha*AtA
    S_{k+1} = relu(Wc S_k - beta*S_{k-1} + B),  Wc = (1+beta)I - alpha*AtA

All iteration weights are constant -> loaded once, no per-iteration streaming.

Device layout (per core, NS=4096 columns): packed [128, 512] per slice s:
partition group g (rows 32g..32g+31) of slice s holds original columns
[g*1024 + 512*s, g*1024 + 512*s + 512).  Weights are diag4 [128,128] blocks so
one full-array matmul advances 4 column blocks; one slice = one PSUM bank.
Per step per slice: 3 accumulating matmuls (ident@B start, Wc@S_k,
(-beta I)@S_{k-1} stop) then relu psum->S (slice 0 on VectorE, slice 1 on
ScalarE) overlapped with the other slice's matmuls.
"""

import os
import sys

import numpy as np

for _p in ("/opt/trn_rl_repo", "/root/.axon_site/_ro/trn_rl_repo"):
    if os.path.isdir(_p) and _p not in sys.path:
        sys.path.append(_p)

from contextlib import ExitStack

import concourse.bass as bass
import concourse.bacc as bacc
import concourse.tile as tile
from concourse import mybir
from concourse.bass_utils import run_bass_kernel_spmd

M, K, N_FULL, N_CORES = 512, 32, 32768, 8
NSTEP = 32               # heavy-ball steps after the warm start

F32 = mybir.dt.float32
F32R = mybir.dt.float32r
F16 = mybir.dt.float16

MM_DTYPE = F32R

LAST_RESULTS = None  # BassKernelResults of the most recent run (for test.py)


def build_program(ns: int, nstep: int, c0: float, alpha: float, mm_dtype=MM_DTYPE):
    """Build the SPMD Bass program for one core holding `ns` columns."""
    DT = mm_dtype
    assert ns == 4096
    SL = 512             # columns per slice (one PSUM bank)
    NSL = 2              # slices

    nc = bacc.Bacc("TRN2", target_bir_lowering=False)

    x_d = nc.dram_tensor("x", [M, ns], F16, kind="ExternalInput")
    apad_d = nc.dram_tensor("apad", [128, 16 * 128], F16, kind="ExternalInput")
    idm_d = nc.dram_tensor("idm", [128, 128], F32, kind="ExternalInput")
    w1_d = nc.dram_tensor("w1", [128, 128], F32, kind="ExternalInput")
    wc_d = nc.dram_tensor("wc", [128, 128], F32, kind="ExternalInput")
    wp_d = nc.dram_tensor("wp", [128, 128], F32, kind="ExternalInput")
    out_d = nc.dram_tensor("s_out", [K, ns], F32, kind="ExternalOutput")

    with ExitStack() as ctx:
        tc = ctx.enter_context(tile.TileContext(nc))
        persist = ctx.enter_context(tc.tile_pool(name="persist", bufs=1))
        psum = ctx.enter_context(tc.tile_pool(name="psum", bufs=4, space="PSUM"))

        # --- X staging first (critical path), slice-major column layout ---
        # xt[c] cols: 2048*s + 512*g + j  <->  x_d col g*1024 + 512*s + j
        xts = [persist.tile([128, ns], F16, name=f"xt{c}") for c in range(4)]
        for c in range(4):
            for s in range(NSL):
                xsrc = x_d[128 * c:128 * (c + 1), :].rearrange(
                    "p (g b j) -> p b g j", g=4, b=NSL, j=SL)[:, s]
                dst = xts[c][:, 2048 * s:2048 * (s + 1)].rearrange(
                    "p (g j) -> p g j", g=4)
                nc.sync.dma_start(dst, xsrc)

        # --- constants on the scalar HWDGE ring (parallel with X) ---
        apc = persist.tile([128, 16 * 128], F16)  # (g,c) chunk at 128*(4g+c)
        nc.scalar.dma_start(apc[:], apad_d[:])
        id_sb = persist.tile([128, 128], DT)
        w1_sb = persist.tile([128, 128], DT)
        wc_sb = persist.tile([128, 128], DT)
        wp_sb = persist.tile([128, 128], DT)
        nc.scalar.dma_start(id_sb[:], idm_d[:].bitcast(DT))
        nc.scalar.dma_start(w1_sb[:], w1_d[:].bitcast(DT))
        nc.scalar.dma_start(wc_sb[:], wc_d[:].bitcast(DT))
        nc.scalar.dma_start(wp_sb[:], wp_d[:].bitcast(DT))

        # --- B = alpha * A.T X  (packed layout), per slice ---
        pb = [psum.tile([128, SL], F32, tag=f"pb{s}", name=f"pb{s}", bufs=1) for s in range(NSL)]
        for c in range(4):
            for s in range(NSL):
                for g in range(4):
                    nc.tensor.matmul(
                        pb[s][:],
                        apc[:, 128 * (4 * g + c):128 * (4 * g + c + 1)],
                        xts[c][:, 2048 * s + SL * g: 2048 * s + SL * (g + 1)],
                        start=(c == 0 and g == 0),
                        stop=(c == 3 and g == 3),
                    )

        b_sb = [persist.tile([128, SL], DT, name=f"b_sb{s}") for s in range(NSL)]
        s_a = [persist.tile([128, SL], DT, name=f"s_a{s}") for s in range(NSL)]
        s_b = [persist.tile([128, SL], DT, name=f"s_b{s}") for s in range(NSL)]

        # drain B = alpha * (A.T X psum) to SBUF, warm-start S0 = relu(c0 * B)
        nc.vector.tensor_scalar_mul(b_sb[0][:], pb[0][:], alpha)
        nc.scalar.activation(b_sb[1][:], pb[1][:],
                             mybir.ActivationFunctionType.Copy, scale=alpha)
        nc.vector.tensor_scalar(s_a[0][:], b_sb[0][:], c0, 0.0,
                                mybir.AluOpType.mult, mybir.AluOpType.max)
        nc.scalar.activation(s_a[1][:], b_sb[1][:],
                             mybir.ActivationFunctionType.Relu, scale=c0)

        # --- heavy-ball loop: step k computes S_{k+1} (S_k in cur) ---
        for k in range(1, nstep + 1):
            cur = s_a if k % 2 == 1 else s_b
            dest = s_b if k % 2 == 1 else s_a
            pts = []
            for s in range(NSL):
                pt = psum.tile([128, SL], F32, tag=f"pt{s}", name=f"pt{s}", bufs=3)
                nc.tensor.matmul(pt[:], id_sb[:], b_sb[s][:],
                                 start=True, stop=False)
                if k == 1:
                    nc.tensor.matmul(pt[:], w1_sb[:], cur[s][:],
                                     start=False, stop=True)
                else:
                    nc.tensor.matmul(pt[:], wp_sb[:], dest[s][:],
                                     start=False, stop=False)
                    nc.tensor.matmul(pt[:], wc_sb[:], cur[s][:],
                                     start=False, stop=True)
                pts.append(pt)
            nc.vector.tensor_scalar_max(dest[0][:], pts[0][:], 0.0)
            nc.scalar.activation(dest[1][:], pts[1][:],
                                 mybir.ActivationFunctionType.Relu)

        final = s_a if nstep % 2 == 0 else s_b
        for s in range(NSL):
            for g in range(4):
                eng = nc.sync if g % 2 == 0 else nc.scalar
                eng.dma_start(
                    out_d[:, g * 1024 + SL * s: g * 1024 + SL * (s + 1)],
                    final[s][K * g:K * (g + 1), :].bitcast(F32),
                )

    nc.finalize()
    return nc


def host_prep(A: np.ndarray, nstep: int):
    """Heavy-ball coefficients + constant device weights from A."""
    A = np.asarray(A, dtype=np.float32)
    AtA = (A.T @ A).astype(np.float64)
    ev = np.linalg.eigvalsh(AtA)
    L, mu = float(ev[-1]), float(ev[0])
    kap = L / mu
    alpha = 4.0 / (np.sqrt(L) + np.sqrt(mu)) ** 2
    beta = ((np.sqrt(kap) - 1.0) / (np.sqrt(kap) + 1.0)) ** 2
    c0 = (1.0 / L) / alpha

    W1 = (np.eye(K) - alpha * AtA)
    Wc = ((1.0 + beta) * np.eye(K) - alpha * AtA)

    def diag4(Wt):
        out = np.zeros((128, 128), dtype=np.float32)
        for g in range(4):
            out[K * g:K * (g + 1), K * g:K * (g + 1)] = Wt.astype(np.float32)
        return out

    w1 = diag4(W1.T)
    wc = diag4(Wc.T)
    wp = (-beta * np.eye(128)).astype(np.float32)
    idm = np.eye(128, dtype=np.float32)

    A16 = A.astype(np.float16)
    apad = np.zeros((128, 16 * 128), dtype=np.float16)
    for g in range(4):
        for c in range(4):
            blk = np.zeros((128, 128), dtype=np.float16)
            blk[:, K * g:K * (g + 1)] = A16[128 * c:128 * (c + 1), :]
            apad[:, 128 * (4 * g + c):128 * (4 * g + c + 1)] = blk
    return apad, idm, w1, wc, wp, float(c0), float(alpha)


_PROGRAM_CACHE = {}


def _get_program(ns, nstep, c0, alpha):
    key = (ns, nstep, round(c0, 10), round(alpha, 12), str(MM_DTYPE))
    if key not in _PROGRAM_CACHE:
        _PROGRAM_CACHE[key] = build_program(ns, nstep, c0, alpha)
    return _PROGRAM_CACHE[key]


def kernel(X: np.ndarray, A: np.ndarray) -> np.ndarray:
    global LAST_RESULTS
    X = np.ascontiguousarray(np.asarray(X, dtype=np.float32))
    A = np.ascontiguousarray(np.asarray(A, dtype=np.float32))
    assert X.shape == (M, N_FULL) and A.shape == (M, K)

    ns = N_FULL // N_CORES
    apad, idm, w1, wc, wp, c0, alpha = host_prep(A, NSTEP)
    nc = _get_program(ns, NSTEP, c0, alpha)

    in_maps = []
    for c in range(N_CORES):
        in_maps.append({
            "x": np.ascontiguousarray(X[:, c * ns:(c + 1) * ns].astype(np.float16)),
            "apad": apad,
            "idm": idm,
            "w1": w1,
            "wc": wc,
            "wp": wp,
        })

    res = run_bass_kernel_spmd(nc, in_maps, core_ids=list(range(N_CORES)))
    LAST_RESULTS = res
    S = np.concatenate([res.results[c]["s_out"] for c in range(N_CORES)], axis=1)
    return np.ascontiguousarray(S.astype(np.float32))
